# revision 2
# baseline (speedup 1.0000x reference)
"""BidirectionalMamba2 on 8 Trainium2 NeuronCores (Bass/Tile kernel).

Sharding: 8 cores = 4 batches x 2 directions (branch parallelism per the
sharding hint). Each core runs one full Mamba2 branch channel-major with the
SSD in position-major chunk pairs; host pre-reverses x for the backward cores
and adds fwd + reversed-bwd outputs at the end.

Self-contained: includes the kernel builder (no sibling imports). The PJRT
executable is cached across calls.
"""
import numpy as np
import ml_dtypes
from contextlib import ExitStack

import concourse.bass as bass
import concourse.mybir as mybir
import concourse.tile as tile

F32 = mybir.dt.float32
F32R = mybir.dt.float32r
BF16 = mybir.dt.bfloat16
AOP = mybir.AluOpType
ACTF = mybir.ActivationFunctionType

CH = 64
HEADDIM = 64
DSTATE = 128
EPS = 1e-5
H = 8
DIN = 512
CONV_CH = 768
DPROJ_PAD = 1296
COUT = 256
BLK = 1024
NG = BLK // 128
NCB = BLK // CH
L_FULL = 8192
B_FULL = 4
CIN = 256


def split_multiwaits(nc):
    """This walrus build rejects >1 sem wait per instruction; split extras
    onto single-wait Drains inserted before the instruction."""
    for fn in nc.m.functions:
        for bb in fn.blocks:
            newinsts = []
            for inst in bb.instructions:
                si = inst.sync_info
                if si is not None and si.on_wait and len(si.on_wait) > 1:
                    waits = list(si.on_wait)
                    for w in waits[:-1]:
                        d = mybir.InstDrain(
                            name=nc.get_next_instruction_name(), ins=[], outs=[])
                        d.engine = inst.engine
                        d.sync_info = mybir.SyncInfo(on_wait=[w], on_update=[])
                        nc.register_instruction(d)
                        newinsts.append(d)
                    si.on_wait = [waits[-1]]
                newinsts.append(inst)
            bb.instructions = newinsts


def build_mamba_nc(L=L_FULL):
    nblk = L // BLK
    nc = bass.Bass(trn_type="TRN2")

    x_d = nc.declare_dram_parameter("x", [2, 128, L], BF16, isOutput=False)
    wfT_d = nc.declare_dram_parameter("wfT", [2, 128, DPROJ_PAD], BF16, isOutput=False)
    convw_d = nc.declare_dram_parameter("convw", [6, 128, 4], F32, isOutput=False)
    convb_d = nc.declare_dram_parameter("convb", [6, 128, 1], F32, isOutput=False)
    dtbias_d = nc.declare_dram_parameter("dtbias", [16, 1], F32, isOutput=False)
    acol_d = nc.declare_dram_parameter("acol", [16, 1], F32, isOutput=False)
    dfull_d = nc.declare_dram_parameter("dfull", [128, DIN], BF16, isOutput=False)
    w2T_d = nc.declare_dram_parameter("w2T", [4, 128, COUT], BF16, isOutput=False)
    ltpair_d = nc.declare_dram_parameter("ltpair", [128, CH], F32R, isOutput=False)
    uts_d = nc.declare_dram_parameter("uts", [128, CH], F32R, isOutput=False)
    utmask_d = nc.declare_dram_parameter("utmask", [128, CH], BF16, isOutput=False)
    y_d = nc.declare_dram_parameter("y", [2, 128, L], F32, isOutput=True)

    with tile.TileContext(nc) as tc:
        with ExitStack() as ctx:
            const = ctx.enter_context(tc.tile_pool(name="const", bufs=1))
            big1 = ctx.enter_context(tc.tile_pool(name="big1", bufs=1))
            big2 = ctx.enter_context(tc.tile_pool(name="big2", bufs=2))
            sm = ctx.enter_context(tc.tile_pool(name="sm", bufs=1))
            scr = ctx.enter_context(tc.tile_pool(name="scr", bufs=2))
            pe512 = ctx.enter_context(tc.tile_pool(name="pe512", bufs=2, space="PSUM"))
            smallp = ctx.enter_context(tc.tile_pool(name="smallp", bufs=1, space="PSUM"))
            dfp = ctx.enter_context(tc.tile_pool(name="dfp", bufs=1, space="PSUM"))
            stpp = ctx.enter_context(tc.tile_pool(name="stpp", bufs=1, space="PSUM"))
            ydpp = ctx.enter_context(tc.tile_pool(name="ydpp", bufs=1, space="PSUM"))
            yopp = ctx.enter_context(tc.tile_pool(name="yopp", bufs=1, space="PSUM"))

            wfT = const.tile(name="wfT", shape=[128, 2, DPROJ_PAD], dtype=BF16)
            nc.sync.dma_start(out=wfT, in_=wfT_d[:, :, :].rearrange("a b c -> b a c"))
            convw = const.tile(name="convw", shape=[128, 6, 4], dtype=F32)
            nc.sync.dma_start(out=convw, in_=convw_d[:, :, :].rearrange("a b c -> b a c"))
            convb = const.tile(name="convb", shape=[128, 6, 1], dtype=F32)
            nc.sync.dma_start(out=convb, in_=convb_d[:, :, :].rearrange("a b c -> b a c"))
            dtbias = const.tile(name="dtbias", shape=[16, 1], dtype=F32)
            nc.sync.dma_start(out=dtbias, in_=dtbias_d[:, :])
            acol = const.tile(name="acol", shape=[16, 1], dtype=F32)
            nc.sync.dma_start(out=acol, in_=acol_d[:, :])
            dfull = const.tile(name="dfull", shape=[128, DIN], dtype=BF16)
            nc.sync.dma_start(out=dfull, in_=dfull_d[:, :])
            w2T = const.tile(name="w2T", shape=[128, 4, COUT], dtype=BF16)
            nc.sync.dma_start(out=w2T, in_=w2T_d[:, :, :].rearrange("a b c -> b a c"))
            ltpair = const.tile(name="ltpair", shape=[128, CH], dtype=F32R)
            nc.sync.dma_start(out=ltpair, in_=ltpair_d[:, :])
            uts = const.tile(name="uts", shape=[128, CH], dtype=F32R)
            nc.sync.dma_start(out=uts, in_=uts_d[:, :])
            utmask = const.tile(name="utmask", shape=[128, CH], dtype=BF16)
            nc.sync.dma_start(out=utmask, in_=utmask_d[:, :])

            maskseg = const.tile(name="maskseg", shape=[16, BLK], dtype=F32)
            nc.vector.memset(maskseg, 1.0)
            nc.gpsimd.affine_select(
                out=maskseg, in_=maskseg, compare_op=AOP.is_gt, fill=0.0,
                base=0, pattern=[[0, NCB], [1, CH]], channel_multiplier=0)
            onestop = const.tile(name="onestop", shape=[1, 128], dtype=F32)
            nc.vector.memset(onestop, 0.0)
            nc.vector.memset(onestop[:, 0:64], 1.0)
            onesbot = const.tile(name="onesbot", shape=[1, 128], dtype=F32)
            nc.vector.memset(onesbot, 0.0)
            nc.vector.memset(onesbot[:, 64:128], 1.0)
            onesfull = const.tile(name="onesfull", shape=[1, 128], dtype=F32)
            nc.vector.memset(onesfull, 1.0)
            one16 = const.tile(name="one16", shape=[16, 1], dtype=F32)
            nc.vector.memset(one16, 1.0)
            epscol = const.tile(name="epscol", shape=[128, 1], dtype=F32)
            nc.vector.memset(epscol, EPS)

            carry = const.tile(name="carry", shape=[DSTATE, H * HEADDIM], dtype=BF16)
            nc.vector.memset(carry, 0.0)
            mtbd_bufs = []
            for i in range(2):
                mb = const.tile(name=f"mtbd{i}", shape=[128, H, 128], dtype=BF16)
                nc.vector.memset(mb.rearrange("p h c -> p (h c)"), 0.0)
                mtbd_bufs.append(mb)

            prev_halo = None
            for blk in range(nblk):
                boff = blk * BLK
                x_blk = big2.tile(name="x_blk", shape=[128, 2, BLK], dtype=BF16)
                nc.sync.dma_start(
                    out=x_blk,
                    in_=x_d[:, :, boff:boff + BLK].rearrange("a b c -> b a c"))

                zbc = big1.tile(name="zbc", shape=[128, 6, 3 + BLK], dtype=BF16)
                zcm = big1.tile(name="zcm", shape=[128, 4, BLK], dtype=BF16)
                dtv_cm = sm.tile(name="dtv_cm", shape=[16, BLK], dtype=F32)

                if prev_halo is None:
                    nc.vector.memset(zbc[:, :, 0:3], 0.0)
                else:
                    nc.vector.tensor_copy(out=zbc[:, :, 0:3], in_=prev_halo)

                for m in range(11):
                    mrows = 128 if m < 10 else 16
                    for n in range(2):
                        ps = pe512.tile(name="ps_zt", shape=[128, 512], dtype=F32,
                                        tag="pe512")
                        for k in range(2):
                            nc.tensor.matmul(
                                ps[0:mrows, :],
                                lhsT=wfT[:, k, m * 128:m * 128 + mrows],
                                rhs=x_blk[:, k, n * 512:(n + 1) * 512],
                                start=(k == 0), stop=(k == 1))
                        cols = slice(n * 512, (n + 1) * 512)
                        if m < 4:
                            nc.scalar.copy(zcm[:, m, cols], ps[:, :])
                        elif m < 10:
                            nc.scalar.copy(
                                zbc[:, m - 4, 3 + n * 512: 3 + (n + 1) * 512],
                                ps[:, :])
                        else:
                            et = scr.tile(name="et", shape=[16, 512], dtype=F32)
                            nc.scalar.activation(out=et, in_=ps[0:16, :],
                                                 func=ACTF.Exp,
                                                 bias=dtbias[:, :], scale=1.0)
                            nc.scalar.activation(out=dtv_cm[:, cols], in_=et,
                                                 func=ACTF.Ln, bias=one16[:, :],
                                                 scale=1.0)

                xbcc = big1.tile(name="xbcc", shape=[128, 6, BLK], dtype=BF16)
                for ct in range(6):
                    acc = scr.tile(name="conv_acc", shape=[128, BLK], dtype=BF16,
                                   tag="conv_acc")
                    nc.vector.tensor_scalar(
                        out=acc, in0=zbc[:, ct, 0:BLK],
                        scalar1=convw[:, ct, 0:1], scalar2=None, op0=AOP.mult)
                    for k in range(1, 4):
                        nc.vector.scalar_tensor_tensor(
                            out=acc, in0=zbc[:, ct, k:k + BLK],
                            scalar=convw[:, ct, k:k + 1], in1=acc,
                            op0=AOP.mult, op1=AOP.add)
                    sg = scr.tile(name="conv_sig", shape=[128, BLK], dtype=BF16,
                                  tag="conv_sig")
                    nc.scalar.activation(out=sg, in_=acc, func=ACTF.Sigmoid,
                                         bias=convb[:, ct, :], scale=1.0)
                    accb = scr.tile(name="conv_accb", shape=[128, BLK], dtype=BF16,
                                    tag="conv_accb")
                    nc.vector.tensor_scalar(
                        out=accb, in0=acc, scalar1=convb[:, ct, :], scalar2=None,
                        op0=AOP.add)
                    nc.gpsimd.tensor_tensor(out=xbcc[:, ct, :], in0=accb, in1=sg,
                                            op=AOP.mult)
                halo = scr.tile(name="halo", shape=[128, 6, 3], dtype=BF16,
                                tag="halo")
                nc.vector.tensor_copy(out=halo, in_=zbc[:, :, BLK:BLK + 3])
                prev_halo = halo

                dtv_bf = sm.tile(name="dtv_bf", shape=[16, BLK], dtype=BF16)
                nc.scalar.copy(dtv_bf, dtv_cm)
                dta_fm = sm.tile(name="dta_fm", shape=[16, BLK], dtype=F32)
                nc.vector.tensor_scalar(out=dta_fm, in0=dtv_cm, scalar1=acol[:, :],
                                        scalar2=None, op0=AOP.mult)
                acum_fm = sm.tile(name="acum_fm", shape=[16, BLK], dtype=F32)
                nc.vector.tensor_tensor_scan(
                    out=acum_fm, data0=maskseg, data1=dta_fm, initial=0.0,
                    op0=AOP.mult, op1=AOP.add)
                dta_hi = sm.tile(name="dta_hi", shape=[16, BLK], dtype=BF16)
                nc.scalar.copy(dta_hi, dta_fm)
                dta_lo = sm.tile(name="dta_lo", shape=[16, BLK], dtype=BF16)
                nc.vector.tensor_tensor(out=dta_lo, in0=dta_fm, in1=dta_hi,
                                        op=AOP.subtract)
                totrow = sm.tile(name="totrow", shape=[1, 256], dtype=F32)
                base = acum_fm[:, CH - 1:CH]
                tot_sv = bass.AP(tensor=base.tensor, offset=base.offset,
                                 ap=[base.ap[0], [CH, NCB]])
                nc.sync.dma_start(
                    out=totrow.rearrange("a (h c) -> a h c", h=16), in_=tot_sv)
                cdrow = sm.tile(name="cdrow", shape=[1, 256], dtype=F32)
                nc.scalar.activation(out=cdrow, in_=totrow, func=ACTF.Exp)
                cdps = smallp.tile(name="cdps", shape=[128, 256], dtype=F32,
                                   tag="smallp")
                nc.tensor.matmul(cdps[:, :], lhsT=onesfull[:, :], rhs=cdrow[:, :],
                                 start=True, stop=True)
                cdcols = sm.tile(name="cdcols", shape=[128, 256], dtype=F32)
                nc.scalar.copy(cdcols, cdps)
                tbps = smallp.tile(name="tbps", shape=[128, NG * 16], dtype=F32,
                                   tag="smallp")
                t_even = bass.AP(tensor=totrow.tensor, offset=totrow.offset,
                                 ap=[totrow.ap[0], [2, NG], [16, 16]])
                sl_odd = totrow[:, 1:2]
                t_odd = bass.AP(tensor=sl_odd.tensor, offset=sl_odd.offset,
                                ap=[sl_odd.ap[0], [2, NG], [16, 16]])
                nc.tensor.matmul(tbps[:, :], lhsT=onestop[:, :], rhs=t_even,
                                 start=True, stop=False)
                nc.tensor.matmul(tbps[:, :], lhsT=onesbot[:, :], rhs=t_odd,
                                 start=False, stop=True)
                tb_sb = sm.tile(name="tb_sb", shape=[128, NG * 16], dtype=F32)
                nc.scalar.copy(tb_sb, tbps)

                xbct = big1.tile(name="xbct", shape=[128, NG, CONV_CH], dtype=BF16)
                for g in range(NG):
                    for ct in range(6):
                        nc.sync.dma_start_transpose(
                            out=xbct[:, g, ct * 128:(ct + 1) * 128],
                            in_=xbcc[:, ct, g * 128:(g + 1) * 128])
                zt_t = big1.tile(name="zt_t", shape=[128, NG, DIN], dtype=BF16)
                for g in range(NG):
                    for ct in range(4):
                        nc.sync.dma_start_transpose(
                            out=zt_t[:, g, ct * 128:(ct + 1) * 128],
                            in_=zcm[:, ct, g * 128:(g + 1) * 128])
                dtv_t = sm.tile(name="dtv_t", shape=[128, NG * 16], dtype=BF16)
                hi_t = sm.tile(name="hi_t", shape=[128, NG * 16], dtype=BF16)
                lo_t = sm.tile(name="lo_t", shape=[128, NG * 16], dtype=BF16)
                for g in range(NG):
                    cols = slice(g * 16, (g + 1) * 16)
                    src = slice(g * 128, (g + 1) * 128)
                    nc.sync.dma_start_transpose(out=dtv_t[:, cols],
                                                in_=dtv_bf[:, src])
                    nc.sync.dma_start_transpose(out=hi_t[:, cols],
                                                in_=dta_hi[:, src])
                    nc.sync.dma_start_transpose(out=lo_t[:, cols],
                                                in_=dta_lo[:, src])
                dta_t = sm.tile(name="dta_t", shape=[128, NG * 16], dtype=F32R)
                nc.vector.tensor_tensor(out=dta_t, in0=hi_t, in1=lo_t, op=AOP.add)

                acps = smallp.tile(name="acps", shape=[64, NG * 16], dtype=F32,
                                   tag="smallp")
                nc.tensor.matmul(acps[:, :], lhsT=ltpair[0:64, :],
                                 rhs=dta_t[0:64, :], start=True, stop=True)
                acum_t = sm.tile(name="acum_t", shape=[128, NG * 16], dtype=F32)
                nc.scalar.copy(acum_t[0:64, :], acps[:, :])
                acps2 = smallp.tile(name="acps2", shape=[64, NG * 16], dtype=F32,
                                    tag="smallp")
                nc.tensor.matmul(acps2[:, :], lhsT=ltpair[64:128, :],
                                 rhs=dta_t[64:128, :], start=True, stop=True)
                nc.scalar.copy(acum_t[64:128, :], acps2[:, :])
                dec_f = sm.tile(name="dec_f", shape=[128, NG * 16], dtype=F32)
                nc.vector.tensor_tensor(out=dec_f, in0=tb_sb, in1=acum_t,
                                        op=AOP.subtract)
                decay_s = sm.tile(name="decay_s", shape=[128, NG * 16], dtype=BF16)
                nc.scalar.activation(out=decay_s, in_=dec_f, func=ACTF.Exp)
                expa = sm.tile(name="expa", shape=[128, NG * 16], dtype=BF16)
                nc.scalar.activation(out=expa, in_=acum_t, func=ACTF.Exp)

                xh_view = xbct[:, :, 0:DIN]
                xs = big1.tile(name="xs", shape=[128, NG, DIN], dtype=BF16)
                dtv_exp = bass.AP(
                    tensor=dtv_t.tensor, offset=dtv_t.offset,
                    ap=[dtv_t.ap[0], [16, NG], [1, H], [0, HEADDIM]])
                nc.vector.tensor_tensor(
                    out=xs.rearrange("p g (h q) -> p g h q", h=H),
                    in0=xh_view.rearrange("p g (h q) -> p g h q", h=H),
                    in1=dtv_exp, op=AOP.mult)
                xdec = big1.tile(name="xdec", shape=[128, NG, DIN], dtype=BF16)
                dec_exp = bass.AP(
                    tensor=decay_s.tensor, offset=decay_s.offset,
                    ap=[decay_s.ap[0], [16, NG], [1, H], [0, HEADDIM]])
                nc.vector.tensor_tensor(
                    out=xdec.rearrange("p g (h q) -> p g h q", h=H),
                    in0=xs.rearrange("p g (h q) -> p g h q", h=H),
                    in1=dec_exp, op=AOP.mult)

                ysb_blk = big1.tile(name="ysb_blk", shape=[128, NG, DIN], dtype=BF16)
                for g in range(NG):
                    b_pos = xbct[:, g, DIN:DIN + DSTATE]
                    b_cm0 = xbcc[:, 4, (2 * g) * CH:(2 * g + 1) * CH]
                    b_cm1 = xbcc[:, 4, (2 * g + 1) * CH:(2 * g + 2) * CH]
                    c_cm0 = xbcc[:, 5, (2 * g) * CH:(2 * g + 1) * CH]
                    c_cm1 = xbcc[:, 5, (2 * g + 1) * CH:(2 * g + 2) * CH]

                    gtps = smallp.tile(name="gtps", shape=[128, CH], dtype=F32,
                                       tag="smallp")
                    nc.tensor.matmul(gtps[0:64, :], lhsT=b_cm0, rhs=c_cm0,
                                     start=True, stop=True)
                    nc.tensor.matmul(gtps[64:128, :], lhsT=b_cm1, rhs=c_cm1,
                                     start=True, stop=True)
                    gtm = scr.tile(name="gtm", shape=[128, CH], dtype=BF16,
                                   tag="gtm")
                    nc.vector.tensor_tensor(out=gtm, in0=gtps, in1=utmask,
                                            op=AOP.mult)

                    tmp = scr.tile(name="tmp", shape=[128, H, CH], dtype=F32R,
                                   tag="tmp")
                    for h in range(H):
                        nc.gpsimd.tensor_scalar(
                            out=tmp[:, h, :], in0=ltpair.bitcast(F32),
                            scalar1=dta_t.bitcast(F32)[:, g * 16 + h:
                                                       g * 16 + h + 1],
                            scalar2=None, op0=AOP.mult)
                    dfps = dfp.tile(name="dfps", shape=[64, H * CH], dtype=F32,
                                    tag="dfps")
                    nc.tensor.matmul(
                        dfps[:, :], lhsT=uts[0:64, :],
                        rhs=tmp[0:64, :, :].rearrange("p h c -> p (h c)"),
                        start=True, stop=True)
                    lmat = scr.tile(name="lmat", shape=[128, H, CH], dtype=BF16,
                                    tag="lmat")
                    nc.scalar.activation(
                        out=lmat[0:64, :, :].rearrange("p h c -> p (h c)"),
                        in_=dfps, func=ACTF.Exp)
                    dfps2 = dfp.tile(name="dfps2", shape=[64, H * CH], dtype=F32,
                                     tag="dfps")
                    nc.tensor.matmul(
                        dfps2[:, :], lhsT=uts[64:128, :],
                        rhs=tmp[64:128, :, :].rearrange("p h c -> p (h c)"),
                        start=True, stop=True)
                    nc.scalar.activation(
                        out=lmat[64:128, :, :].rearrange("p h c -> p (h c)"),
                        in_=dfps2, func=ACTF.Exp)

                    mtbd = mtbd_bufs[(blk * NG + g) % 2]
                    g_top = gtm[0:64, :]
                    g_bot = gtm[64:128, :]
                    nc.vector.tensor_tensor(
                        out=mtbd[0:64, :, 0:CH], in0=lmat[0:64, :, :],
                        in1=bass.AP(tensor=g_top.tensor, offset=g_top.offset,
                                    ap=[g_top.ap[0], [0, H], g_top.ap[1]]),
                        op=AOP.mult)
                    nc.vector.tensor_tensor(
                        out=mtbd[64:128, :, CH:128], in0=lmat[64:128, :, :],
                        in1=bass.AP(tensor=g_bot.tensor, offset=g_bot.offset,
                                    ap=[g_bot.ap[0], [0, H], g_bot.ap[1]]),
                        op=AOP.mult)

                    ydps = ydpp.tile(name="ydps", shape=[128, DIN], dtype=F32)
                    for h in range(H):
                        nc.tensor.matmul(
                            ydps[:, h * HEADDIM:(h + 1) * HEADDIM],
                            lhsT=mtbd[:, h, :],
                            rhs=xs[:, g, h * HEADDIM:(h + 1) * HEADDIM],
                            start=True, stop=True)

                    yops = yopp.tile(name="yops", shape=[128, DIN], dtype=F32)
                    for par in range(2):
                        c_loc = 2 * g + par
                        rows = slice(par * 64, (par + 1) * 64)
                        stp = stpp.tile(name="stp", shape=[DSTATE, DIN], dtype=F32)
                        nc.tensor.matmul(stp[:, :], lhsT=b_pos[rows, :],
                                         rhs=xdec[rows, g, :],
                                         start=True, stop=True)
                        c_cm = c_cm0 if par == 0 else c_cm1
                        nc.tensor.matmul(yops[rows, :], lhsT=c_cm,
                                         rhs=carry[:, :], start=True, stop=True)
                        for h in range(H):
                            hs = slice(h * HEADDIM, (h + 1) * HEADDIM)
                            nc.vector.scalar_tensor_tensor(
                                out=carry[:, hs], in0=carry[:, hs],
                                scalar=cdcols[:, h * 16 + c_loc:
                                              h * 16 + c_loc + 1],
                                in1=stp[:, hs], op0=AOP.mult, op1=AOP.add)

                    ysb_g = ysb_blk[:, g, :]
                    ea_sl = expa[:, g * 16: g * 16 + H]
                    nc.vector.tensor_tensor(
                        out=ysb_g.rearrange("p (h q) -> p h q", h=H),
                        in0=yops.rearrange("p (h q) -> p h q", h=H),
                        in1=bass.AP(tensor=ea_sl.tensor, offset=ea_sl.offset,
                                    ap=[ea_sl.ap[0], [1, H], [0, HEADDIM]]),
                        op=AOP.mult)
                    nc.vector.tensor_tensor(out=ysb_g, in0=ysb_g, in1=ydps,
                                            op=AOP.add)

                xhd = big1.tile(name="xhd", shape=[128, NG, DIN], dtype=BF16)
                nc.gpsimd.tensor_tensor(
                    out=xhd, in0=xh_view,
                    in1=bass.AP(tensor=dfull.tensor, offset=dfull.offset,
                                ap=[dfull.ap[0], [0, NG], dfull.ap[1]]),
                    op=AOP.mult)
                nc.vector.tensor_tensor(out=ysb_blk, in0=ysb_blk, in1=xhd,
                                        op=AOP.add)
                zsig = big1.tile(name="zsig", shape=[128, NG, DIN], dtype=BF16)
                nc.scalar.activation(out=zsig, in_=zt_t, func=ACTF.Sigmoid)
                nc.gpsimd.tensor_tensor(out=zsig, in0=zt_t, in1=zsig, op=AOP.mult)
                nc.vector.tensor_tensor(out=ysb_blk, in0=ysb_blk, in1=zsig,
                                        op=AOP.mult)
                ssum = sm.tile(name="ssum", shape=[128, NG], dtype=F32)
                for g in range(NG):
                    nc.scalar.activation(
                        out=xhd[:, g, :], in_=ysb_blk[:, g, :], func=ACTF.Square,
                        accum_out=ssum[:, g:g + 1])
                rms = sm.tile(name="rms", shape=[128, NG], dtype=F32)
                nc.scalar.activation(out=rms, in_=ssum, func=ACTF.Sqrt,
                                     bias=epscol[:, :], scale=1.0 / DIN)
                rinv = sm.tile(name="rinv", shape=[128, NG], dtype=F32)
                nc.vector.reciprocal(out=rinv, in_=rms)
                nc.vector.tensor_tensor(
                    out=ysb_blk, in0=ysb_blk,
                    in1=bass.AP(tensor=rinv.tensor, offset=rinv.offset,
                                ap=[rinv.ap[0], [1, NG], [0, DIN]]),
                    op=AOP.mult)

                ycm = big1.tile(name="ycm", shape=[128, 4, BLK], dtype=BF16,
                                tag="zcm")
                for g in range(NG):
                    for ct in range(4):
                        nc.sync.dma_start_transpose(
                            out=ycm[:, ct, g * 128:(g + 1) * 128],
                            in_=ysb_blk[:, g, ct * 128:(ct + 1) * 128])
                for mo in range(2):
                    for no in range(2):
                        ops_ = pe512.tile(name="ops_", shape=[128, 512],
                                          dtype=F32, tag="pe512")
                        for kc in range(4):
                            nc.tensor.matmul(
                                ops_[:, :],
                                lhsT=w2T[:, kc, mo * 128:(mo + 1) * 128],
                                rhs=ycm[:, kc, no * 512:(no + 1) * 512],
                                start=(kc == 0), stop=(kc == 3))
                        osb = scr.tile(name="osb", shape=[128, 512], dtype=F32,
                                       tag="osb")
                        nc.scalar.copy(osb, ops_)
                        nc.sync.dma_start(
                            out=y_d[mo, :, boff + no * 512: boff + (no + 1) * 512],
                            in_=osb)

    split_multiwaits(nc)
    return nc


# ================= host side =================

def fold_weights(inputs, pre):
    f = lambda k: np.asarray(inputs[pre + "_" + k], np.float32)
    Wfc_in = np.asarray(inputs["W_fc_in"], np.float32)
    Wfc_out = np.asarray(inputs["W_fc_out"], np.float32)
    Win, convw, convb = f("Win"), f("convw"), f("convb")
    Alog, D, dtbias, normw, Wout = (f("Alog"), f("D"), f("dtbias"),
                                    f("normw"), f("Wout"))
    wf = Win @ Wfc_in
    wf = np.concatenate([wf, np.zeros((DPROJ_PAD - wf.shape[0], wf.shape[1]),
                                      np.float32)], 0)
    wfT = np.ascontiguousarray(wf.T)
    w2T = np.ascontiguousarray((Wfc_out @ Wout @ np.diag(normw)).T)

    A = -np.exp(Alog)
    acol = np.zeros((16, 1), np.float32); acol[:8, 0] = A
    dtb = np.zeros((16, 1), np.float32); dtb[:8, 0] = dtbias
    dfull = np.repeat(D, HEADDIM)[None, :].repeat(128, 0)

    j = np.arange(CH)
    lt = (j[:, None] <= j[None, :]).astype(np.float32)
    uts1 = (j[:, None] > j[None, :]).astype(np.float32)
    utm = (j[:, None] <= j[None, :]).astype(np.float32)

    bf = ml_dtypes.bfloat16
    return {
        "wfT": np.ascontiguousarray(wfT.reshape(2, 128, DPROJ_PAD)).astype(bf),
        "convw": np.ascontiguousarray(convw.reshape(6, 128, 4)),
        "convb": np.ascontiguousarray(convb.reshape(6, 128, 1)),
        "dtbias": dtb, "acol": acol,
        "dfull": dfull.astype(bf),
        "w2T": np.ascontiguousarray(w2T.reshape(4, 128, COUT)).astype(bf),
        "ltpair": np.concatenate([lt, lt], 0),
        "uts": np.concatenate([uts1, uts1], 0),
        "utmask": np.concatenate([utm, utm], 0).astype(bf),
    }


def make_in_maps(inputs, L=L_FULL):
    bf = ml_dtypes.bfloat16
    x = np.asarray(inputs["x"], np.float32)
    wf = fold_weights(inputs, "f")
    wb = fold_weights(inputs, "b")
    maps = []
    for b in range(B_FULL):
        m = dict(wf)
        m["x"] = np.ascontiguousarray(x[b].reshape(2, 128, L)).astype(bf)
        maps.append(m)
    for b in range(B_FULL):
        m = dict(wb)
        m["x"] = np.ascontiguousarray(x[b, :, ::-1].reshape(2, 128, L)).astype(bf)
        maps.append(m)
    return maps


def combine_results(results, L=L_FULL):
    out = np.empty((B_FULL, COUT, L), np.float32)
    for b in range(B_FULL):
        yf = np.asarray(results[b]["y"]).reshape(COUT, L)
        yb = np.asarray(results[b + 4]["y"]).reshape(COUT, L)
        out[b] = yf + yb[:, ::-1]
    return out


# ================= cached PJRT runner =================

_RUNNER = None


class _Runner:
    """Compiles the SPMD NEFF once and keeps a jitted shard_map callable."""

    def __init__(self, L=L_FULL):
        import jax
        from jax.sharding import Mesh, PartitionSpec
        from jax.experimental.shard_map import shard_map
        from concourse import bass2jax

        self.nc = build_mamba_nc(L=L)
        nc = self.nc
        bass2jax.install_neuronx_cc_hook()

        partition_name = (nc.partition_id_tensor.name
                          if nc.partition_id_tensor else None)
        in_names, out_names, out_avals, zero_outs = [], [], [], []
        for alloc in nc.m.functions[0].allocations:
            if not isinstance(alloc, mybir.MemoryLocationSet):
                continue
            name = alloc.memorylocations[0].name
            if alloc.kind == "ExternalInput":
                if name != partition_name:
                    in_names.append(name)
            elif alloc.kind == "ExternalOutput":
                shape = tuple(alloc.tensor_shape)
                dtype = mybir.dt.np(alloc.dtype)
                out_names.append(name)
                out_avals.append(jax.core.ShapedArray(shape, dtype))
                zero_outs.append(np.zeros(shape, dtype))
        self.in_names = list(in_names)
        self.out_names = out_names
        self.out_avals = out_avals
        self.zero_outs = zero_outs
        n_params = len(in_names)
        n_outs = len(out_avals)
        all_in = in_names + out_names + ([partition_name] if partition_name else [])
        donate = tuple(range(n_params, n_params + n_outs))

        def _body(*args):
            operands = list(args)
            if partition_name is not None:
                operands.append(bass2jax.partition_id_tensor())
            outs = bass2jax._bass_exec_p.bind(
                *operands,
                out_avals=tuple(out_avals),
                in_names=tuple(all_in),
                out_names=tuple(out_names),
                lowering_input_output_aliases=(),
                sim_require_finite=True,
                sim_require_nnan=True,
                nc=nc,
            )
            return tuple(outs)

        devices = jax.devices()[:8]
        mesh = Mesh(np.asarray(devices), ("core",))
        in_specs = (PartitionSpec("core"),) * (n_params + n_outs)
        out_specs = (PartitionSpec("core"),) * n_outs
        self.fn = jax.jit(
            shard_map(_body, mesh=mesh, in_specs=in_specs, out_specs=out_specs,
                      check_rep=False),
            donate_argnums=donate, keep_unused=True)
        self.n_params = n_params

    def __call__(self, in_maps):
        concat_in = [
            np.concatenate([np.asarray(in_maps[c][nm]) for c in range(8)], axis=0)
            for nm in self.in_names
        ]
        concat_zeros = [
            np.zeros((8 * z.shape[0], *z.shape[1:]), z.dtype)
            for z in self.zero_outs
        ]
        out_arrs = self.fn(*concat_in, *concat_zeros)
        return [
            {nm: np.asarray(out_arrs[i]).reshape(8, *self.out_avals[i].shape)[c]
             for i, nm in enumerate(self.out_names)}
            for c in range(8)
        ]


def kernel(**inputs):
    global _RUNNER
    if _RUNNER is None:
        _RUNNER = _Runner(L=L_FULL)
    in_maps = make_in_maps(inputs, L_FULL)
    results = _RUNNER(in_maps)
    return combine_results(results, L_FULL)
